# revision 6
# baseline (speedup 1.0000x reference)
"""Cross-attention Trainium2 kernel (self-contained).

Reference computation (B=4, N=M=2048, DIM=1024, H=16, Dh=64):
    q = x @ Wq.T ; k = ctx @ Wk.T ; v = ctx @ Wv.T       (per-head split)
    out = softmax(q k^T / sqrt(Dh)) v                     (per b, h)
    final = out @ Wo.T + bo

Sharding over 8 NeuronCores: core c -> (batch b = c//2, head-group g = c%2).
Each core handles 8 heads (512 of the 1024 inner dims) of one batch and
produces a partial (2048, 1024) output-projection contribution; the host sums
the two partials per batch and adds the bias.

Engine-balance design (per core): ScalarE must run 256 exp instructions of
(1024+352)/1.2 ~= 1147 ns each = 294 us and is the bottleneck; the PE's
~298 us of matmul work (scores pairs on row-groups {0,64} stream
concurrently at ~285 ns/pair; AV K=128 pairs 440 ns; projections) is
paced underneath it as a 256-iteration software pipeline:

    iteration g:  [AV pair for g-2] [~2 paced projection MMs]
                  [scores pair for g+2] and ACT gets [exp g+1].

exp outputs park in an 8-deep SBUF pool so AV never back-pressures
ScalarE; AV accumulators spill psum->SBUF immediately after their last
accumulation so the 2 psum banks recycle within ~1 iteration; softmax
denominators ride along as a ones-column in V (row 64 of the psum
accumulator), normalization (reciprocal+broadcast+mul) runs on DVE/GpSimd
off the critical path.  PSUM budget: scores 2x[128,1024] (4 banks) +
oo 2x[65,512] (2) + projections 2x[128,512] (2) = 8 banks exactly.
"""

import numpy as np
import ml_dtypes
from contextlib import ExitStack

import concourse.bass as bass
import concourse.bacc as bacc
import concourse.tile as tile
from concourse import mybir
from concourse import bass_utils

F32 = mybir.dt.float32
BF16 = mybir.dt.bfloat16

B, N, M, DIM = 4, 2048, 2048, 1024
H, DH = 16, 64
NCORES = 8
HG = DIM // 2          # head dims per core (8 heads * 64)
SCALE = DH ** -0.5

NT = N // 512          # q-row tiles of 512 per head-pair
MT = M // 128          # context-row tiles of 128
CT = DIM // 128        # contraction tiles for projections
DT = HG // 128         # head-pair tiles per core

_CACHE = {}


def _build_program():
    nc = bacc.Bacc(
        "TRN2",
        target_bir_lowering=False,
        debug=False,
        enable_asserts=False,
        num_devices=NCORES,
    )
    xT = nc.dram_tensor("xT", (DIM, N), BF16, kind="ExternalInput").ap()
    ctxT = nc.dram_tensor("ctxT", (DIM, M), BF16, kind="ExternalInput").ap()
    wqT = nc.dram_tensor("wqT", (DIM, HG), BF16, kind="ExternalInput").ap()
    wkT = nc.dram_tensor("wkT", (DIM, HG), BF16, kind="ExternalInput").ap()
    wvT = nc.dram_tensor("wvT", (DIM, HG), BF16, kind="ExternalInput").ap()
    woT = nc.dram_tensor("woT", (HG, DIM), BF16, kind="ExternalInput").ap()
    out = nc.dram_tensor("out", (N, DIM), F32, kind="ExternalOutput").ap()

    with tile.TileContext(nc) as tc:
        _kernel_body(tc, xT, ctxT, wqT, wkT, wvT, woT, out)
    nc.compile()
    return nc


def _kernel_body(tc, xT, ctxT, wqT, wkT, wvT, woT, out):
    nc = tc.nc
    EXP = mybir.ActivationFunctionType.Exp

    with ExitStack() as ctx:
        sb = ctx.enter_context(tc.tile_pool(name="sb", bufs=1))

        xT_sb = sb.tile([128, CT, N], BF16, tag="xT")
        ctxT_sb = sb.tile([128, CT, M], BF16, tag="ctxT")
        wq_sb = sb.tile([128, CT, HG], BF16, tag="wq")
        wk_sb = sb.tile([128, CT, HG], BF16, tag="wk")
        wv_sb = sb.tile([128, CT, HG], BF16, tag="wv")
        wo_sb = sb.tile([128, DT, DIM], BF16, tag="wo")
        qT_sb = sb.tile([128, DT, N], BF16, tag="qT")
        kT_sb = sb.tile([128, DT, M], BF16, tag="kT")
        v_sb = sb.tile([128, MT, 8 * 65], BF16, tag="v")
        on_sb = sb.tile([128, DT, N], BF16, tag="on")

        # ---- loads, ordered so q(0,0)/k(0,0)/v(0..) can start ASAP:
        # pr0 slices of wq/wk first, remaining head-pair columns later ----
        for c in range(CT):
            nc.sync.dma_start(out=wq_sb[:, c, 0:128],
                              in_=wqT[c * 128:(c + 1) * 128, 0:128])
            nc.sync.dma_start(out=xT_sb[:, c, 0:512],
                              in_=xT[c * 128:(c + 1) * 128, 0:512])
        for c in range(CT):
            nc.sync.dma_start(out=wk_sb[:, c, 0:128],
                              in_=wkT[c * 128:(c + 1) * 128, 0:128])
            nc.sync.dma_start(out=ctxT_sb[:, c, 0:512],
                              in_=ctxT[c * 128:(c + 1) * 128, 0:512])
        for c in range(CT):
            nc.sync.dma_start(out=wv_sb[:, c, :], in_=wvT[c * 128:(c + 1) * 128, :])
        for c in range(CT):
            nc.sync.dma_start(out=ctxT_sb[:, c, 512:1024],
                              in_=ctxT[c * 128:(c + 1) * 128, 512:1024])
        for c in range(CT):
            nc.sync.dma_start(out=wk_sb[:, c, 128:512],
                              in_=wkT[c * 128:(c + 1) * 128, 128:512])
        for c in range(CT):
            nc.sync.dma_start(out=wq_sb[:, c, 128:512],
                              in_=wqT[c * 128:(c + 1) * 128, 128:512])
        for c in range(CT):
            nc.sync.dma_start(out=xT_sb[:, c, 512:2048],
                              in_=xT[c * 128:(c + 1) * 128, 512:2048])
        for c in range(CT):
            nc.sync.dma_start(out=ctxT_sb[:, c, 1024:2048],
                              in_=ctxT[c * 128:(c + 1) * 128, 1024:2048])
        for t in range(DT):
            nc.sync.dma_start(out=wo_sb[:, t, :], in_=woT[t * 128:(t + 1) * 128, :])

        v_r = v_sb.rearrange("p m (h x) -> p m h x", x=65)
        for h in range(8):
            nc.vector.memset(v_r[:, :, h, 64:65], 1.0)
        ones1 = sb.tile([1, 64], F32, tag="ones1")
        nc.vector.memset(ones1, 1.0)
        dum = sb.tile([128, 64], BF16, tag="dum")
        nc.vector.memset(dum, 0.0)

        # ---- HAM warmup: ~48 dependency-free tiny matmuls keep the PE
        # active from t~0 so the DMA-gated prologue runs at 2.4 GHz.
        # Pool is closed before the main PSUM pools are allocated. ----
        with tc.tile_pool(name="pw", bufs=2, space="PSUM") as pw, \
             tc.tile_pool(name="sbw", bufs=1) as sbw:
            wps = pw.tile([64, 64], F32, tag="warm", name="wps")
            for r in range(48):
                if r % 24 == 0:
                    wps = pw.tile([64, 64], F32, tag="warm", name="wps")
                nc.tensor.matmul(wps, dum[:, 0:64], dum, start=True, stop=True)
            wsink = sbw.tile([64, 64], F32, tag="wsink", name="wsink")
            nc.vector.tensor_copy(wsink, wps)

        # ---- pools ----
        pss = ctx.enter_context(tc.tile_pool(name="pss", bufs=2, space="PSUM"))
        pso = ctx.enter_context(tc.tile_pool(name="pso", bufs=2, space="PSUM"))
        psp = ctx.enter_context(tc.tile_pool(name="psp", bufs=2, space="PSUM"))
        sba = ctx.enter_context(tc.tile_pool(name="sba", bufs=8))
        sbsp = ctx.enter_context(tc.tile_pool(name="sbsp", bufs=2))
        sbn = ctx.enter_context(tc.tile_pool(name="sbn", bufs=2))
        sbo = ctx.enter_context(tc.tile_pool(name="sbo", bufs=3))

        # ---- paced projection groups (yield every ~2 matmuls) ----
        def q_group(pr, jn):
            ps = psp.tile([128, 512], F32, tag="proj", name="qg")
            for c in range(CT):
                nc.tensor.matmul(
                    ps,
                    wq_sb[:, c, pr * 128:(pr + 1) * 128],
                    xT_sb[:, c, jn * 512:(jn + 1) * 512],
                    start=(c == 0), stop=(c == CT - 1),
                )
                if c % 2 == 1 and c < CT - 1:
                    yield
            nc.vector.tensor_copy(qT_sb[:, pr, jn * 512:(jn + 1) * 512], ps)

        def k_group(pr, jm):
            ps = psp.tile([128, 512], F32, tag="proj", name="kg")
            for c in range(CT):
                nc.tensor.matmul(
                    ps,
                    wk_sb[:, c, pr * 128:(pr + 1) * 128],
                    ctxT_sb[:, c, jm * 512:(jm + 1) * 512],
                    start=(c == 0), stop=(c == CT - 1),
                )
                if c % 2 == 1 and c < CT - 1:
                    yield
            nc.vector.tensor_copy(kT_sb[:, pr, jm * 512:(jm + 1) * 512], ps)

        def v_group(i):
            # all 8 heads (512 projection dims) for ctx tile i in one chain
            ps = psp.tile([128, 512], F32, tag="proj", name="vg")
            for c in range(CT):
                nc.tensor.matmul(
                    ps,
                    ctxT_sb[:, c, i * 128:(i + 1) * 128],
                    wv_sb[:, c, 0:512],
                    start=(c == 0), stop=(c == CT - 1),
                )
                if c % 2 == 1 and c < CT - 1:
                    yield
            nc.vector.tensor_copy(
                v_r[:, i, 0:8, 0:64],
                ps.rearrange("p (h d) -> p h d", h=8),
            )

        def final_group(n128, e):
            ps = psp.tile([128, 512], F32, tag="proj", name="fg")
            for t in range(DT):
                nc.tensor.matmul(
                    ps,
                    on_sb[:, t, n128 * 128:(n128 + 1) * 128],
                    wo_sb[:, t, e * 512:(e + 1) * 512],
                    start=(t == 0), stop=(t == DT - 1),
                )
                if t == 1:
                    yield
            of = sbo.tile([128, 512], F32, tag="of", name="of")
            nc.vector.tensor_copy(of, ps)
            nc.sync.dma_start(
                out=out[n128 * 128:(n128 + 1) * 128, e * 512:(e + 1) * 512],
                in_=of,
            )

        class Pacer:
            def __init__(self):
                self.queue = []
                self.cur = None

            def step(self, n=1):
                # emit up to n chunks (a chunk = segment up to next yield)
                for _ in range(n):
                    while True:
                        if self.cur is None:
                            if not self.queue:
                                return
                            self.cur = self.queue.pop(0)
                        try:
                            next(self.cur)
                            break
                        except StopIteration:
                            self.cur = None

            def drain(self):
                while self.cur is not None or self.queue:
                    self.step()

        pacer = Pacer()

        # ---- attention pipeline pieces ----
        ITERS = [(pr, j, i) for pr in range(DT) for j in range(NT)
                 for i in range(MT)]
        G = len(ITERS)
        sc_tiles = {}
        a_tiles = {}
        oo_of = {}

        def emit_scores(g):
            pr, j, i = ITERS[g]
            s = pss.tile([128, 1024], F32, tag="sc", name="sc")
            for half in range(2):
                lo, hi = half * 64, half * 64 + 64
                nc.tensor.matmul(
                    s[:, half * 512:(half + 1) * 512],
                    kT_sb[lo:hi, pr, i * 128:(i + 1) * 128],
                    qT_sb[lo:hi, pr, j * 512:(j + 1) * 512],
                    start=True, stop=True,
                )
            sc_tiles[g] = s

        def emit_exp(g):
            a = sba.tile([128, 1024], BF16, tag="attn", name="attn")
            nc.scalar.activation(a, sc_tiles.pop(g), EXP, scale=SCALE)
            a_tiles[g] = a

        def emit_norm(pr, j, last=False):
            # spill the two [65,512] psum accumulators to SBUF right away
            # (frees the psum banks), then normalize from the spill.
            # The denominator row is copied to a fresh [1,512] tile so the
            # reciprocal / partition_broadcast see base partition 0.
            oo = oo_of.pop((pr, j))
            sps, dens = [], []
            for half in range(2):
                t = sbsp.tile([64, 512], F32, tag="sp", name="sp")
                nc.vector.tensor_copy(t, oo[half][0:64, :])
                den = sbn.tile([1, 512], F32, tag="den", name="den")
                nc.vector.tensor_copy(den, oo[half][64:65, :])
                sps.append(t)
                dens.append(den)
            bcs = []
            for half in range(2):
                rec32 = sbn.tile([1, 512], F32, tag="rec32", name="rec32")
                nc.vector.reciprocal_approx_fast(out=rec32, in_=dens[half])
                if last:
                    # rank-1 PE broadcast: faster than the GpSimd chain and
                    # keeps the PE warm going into the tail finals
                    bc = psp.tile([64, 512], F32, tag="proj", name="bcp")
                    nc.tensor.matmul(bc, ones1, rec32, start=True, stop=True)
                else:
                    bc = sbn.tile([64, 512], F32, tag="bc", name="bc")
                    nc.gpsimd.partition_broadcast(bc, rec32)
                bcs.append(bc)
            for half in range(2):
                nc.vector.tensor_mul(
                    on_sb[half * 64:half * 64 + 64, pr, j * 512:(j + 1) * 512],
                    sps[half], bcs[half],
                )

        def emit_av(g):
            pr, j, i = ITERS[g]
            if i == 0:
                oo_of[(pr, j)] = [
                    pso.tile([65, 512], F32, tag="oacc", name=f"oacc{h}")
                    for h in range(2)]
            oo = oo_of[(pr, j)]
            a = a_tiles.pop(g)
            for half in range(2):
                nc.tensor.matmul(
                    oo[half],
                    v_r[:, i, 2 * pr + half, :],
                    a[:, half * 512:(half + 1) * 512],
                    start=(i == 0), stop=(i == MT - 1),
                )
            if i == MT - 1:
                emit_norm(pr, j, last=(g == G - 1))

        # ---- upfront (serial) minimal prologue ----
        for gen in [q_group(0, 0), k_group(0, 0),
                    v_group(0), v_group(1), v_group(2)]:
            for _ in gen:
                pass

        # ---- pacing queue; consumed in order ----
        pacer.queue.extend(
            [v_group(3), k_group(0, 1), v_group(4), v_group(5),
             k_group(0, 2), v_group(6), v_group(7), k_group(0, 3),
             v_group(8), v_group(9), q_group(0, 1),
             v_group(10), v_group(11), v_group(12), v_group(13),
             v_group(14), v_group(15), q_group(0, 2), q_group(0, 3)]
            + [g for jn in range(NT)
               for g in (q_group(1, jn), k_group(1, jn))])

        # ---- main pipeline ----
        emit_scores(0)
        emit_scores(1)
        emit_exp(0)
        for g in range(G):
            pr, j, i = ITERS[g]
            if g == 64:   # pr1 starts: queue pr2 q/k
                pacer.queue.extend(
                    [g2 for jn in range(NT)
                     for g2 in (q_group(2, jn), k_group(2, jn))])
            if g == 128:  # pr2 starts: queue pr3 q/k
                pacer.queue.extend(
                    [g2 for jn in range(NT)
                     for g2 in (q_group(3, jn), k_group(3, jn))])
            if pr == 3 and i == 1 and j >= 1:
                # normalize(3, j-1) was just emitted (inside emit_av of
                # (3, j-1, 15) at the previous iteration); queue its finals
                jj = j - 1
                pacer.queue.extend(
                    final_group(n128, e)
                    for n128 in range(jj * 4, jj * 4 + 4)
                    for e in range(2))
            if g + 2 < G:
                emit_scores(g + 2)
            if g >= 2:
                emit_av(g - 2)
            pacer.step(4 if (pr == 0 and j == 0) else 1)
            if g + 1 < G:
                emit_exp(g + 1)
        # tail: last two AV pairs (emits normalize(3,3)), leftover pacing,
        # then the final output projection of the last row block
        emit_av(G - 2)
        emit_av(G - 1)
        pacer.drain()
        for n128 in range(12, 16):
            for e in range(2):
                for _ in final_group(n128, e):
                    pass


def kernel(x, context, Wq, Wk, Wv, Wo, bo):
    x = np.asarray(x, dtype=np.float32)
    context = np.asarray(context, dtype=np.float32)
    Wq = np.asarray(Wq, dtype=np.float32)
    Wk = np.asarray(Wk, dtype=np.float32)
    Wv = np.asarray(Wv, dtype=np.float32)
    Wo = np.asarray(Wo, dtype=np.float32)
    bo = np.asarray(bo, dtype=np.float32)

    if "nc" not in _CACHE:
        _CACHE["nc"] = _build_program()
    nc = _CACHE["nc"]

    in_maps = _make_in_maps(x, context, Wq, Wk, Wv, Wo)
    res = bass_utils.run_bass_kernel_spmd(nc, in_maps, core_ids=list(range(NCORES)))

    final = np.empty((B, N, DIM), dtype=np.float32)
    for b in range(B):
        final[b] = res.results[2 * b]["out"] + res.results[2 * b + 1]["out"] + bo
    return final


def _make_in_maps(x, context, Wq, Wk, Wv, Wo):
    bf = ml_dtypes.bfloat16
    xT = [np.ascontiguousarray(x[b].T).astype(bf) for b in range(B)]
    ctxT = [np.ascontiguousarray(context[b].T).astype(bf) for b in range(B)]
    wT = {}
    for g in range(2):
        sl = slice(g * HG, (g + 1) * HG)
        wT[g] = {
            "wqT": np.ascontiguousarray(Wq[sl, :].T).astype(bf),
            "wkT": np.ascontiguousarray(Wk[sl, :].T).astype(bf),
            "wvT": np.ascontiguousarray(Wv[sl, :].T).astype(bf),
            "woT": np.ascontiguousarray(Wo[:, sl].T).astype(bf),
        }
    in_maps = []
    for c in range(NCORES):
        b, g = c // 2, c % 2
        m = {"xT": xT[b], "ctxT": ctxT[b]}
        m.update(wT[g])
        in_maps.append(m)
    return in_maps


def timed_run(inp, trace_dir=None):
    """Run with NTFF tracing; returns HW exec time in ns (or None)."""
    if "nc" not in _CACHE:
        _CACHE["nc"] = _build_program()
    nc = _CACHE["nc"]
    in_maps = _make_in_maps(
        np.asarray(inp["x"], np.float32), np.asarray(inp["context"], np.float32),
        np.asarray(inp["Wq"], np.float32), np.asarray(inp["Wk"], np.float32),
        np.asarray(inp["Wv"], np.float32), np.asarray(inp["Wo"], np.float32))
    res = bass_utils.run_bass_kernel_spmd(
        nc, in_maps, core_ids=list(range(NCORES)), trace=True, tmpdir=trace_dir)
    return res.exec_time_ns
